# revision 37
# baseline (speedup 1.0000x reference)
"""Trainium2 Bass kernel for nn_ActorNetwork (GNN message passing actor).

Self-contained: hardcodes shapes B=32, K=64, D=4, DS=4, H=512, HH=256, NA=2.
Data-parallel over batch across 8 NeuronCores (4 samples/core), all params
replicated. Returns (mu, std) like the reference.

Host-side weight prep folds: LayerNorm affine into n1, the /counts
normalizations into the LN eps and the head-1 weights, n2 bias into the
head-1 biases. The edge-MLP first layer is split into U = A@o_i + b and
V = C@o_j so the [K,K,2D] edge input tensor is never materialized:
h1[:, (i,j)] = relu(V[:, j] + U[:, i]) via per-i tensor_scalar adds.

Final softplus+clip for std runs on host (ACT func-set limits: softplus
is not co-loadable with sqrt).
"""
import os as _os

import numpy as np

import concourse.bass as bass
import concourse.mybir as mybir
from concourse.bass_utils import run_bass_kernel_spmd
from concourse.tile import TileContext

# ---- problem constants ----
B, K, D, DS, H, HH, NA = 32, 64, 4, 4, 512, 256, 2
NCORES = 8
BSH = B // NCORES            # samples per core = 4
P = 128
FT = H // P                  # 4 feature tiles of hidden dim
TOK = BSH * K                # 256 node tokens per core
IBLK = 8                     # i-rows per edge chunk (8*64 = 512 tokens)
NCH = K // IBLK              # 8 chunks per sample
EPS_S = (K * K) * 1e-5       # LN eps scaled for un-normalized agg sums

F32 = mybir.dt.float32
BF16 = mybir.dt.bfloat16
AF = mybir.ActivationFunctionType
AX = mybir.AxisListType
ALU = mybir.AluOpType

EDGE_DT = _os.environ.get("KERNEL_EDGE_DT", "bf16")   # 'f32' | 'bf16'
SMALL_DT = _os.environ.get("KERNEL_SMALL_DT", "bf16")  # 'f32' | 'bf16'
# how many of the 4 L3 evictions per chunk go to DVE (rest on ACT)
N_EVICT_DVE = int(_os.environ.get("KERNEL_EVICT_DVE", "0"))
# L3 f-tiles whose evict+j-sum run as fused per-i ACT ops (offloads DVE)
N_FUSED_ACT = int(_os.environ.get("KERNEL_FUSED_ACT", "0"))
# h1 style: 'ts1' = single-op TS add + tensor_relu, 'ts2' = fused dual-op TS
H1_STYLE = _os.environ.get("KERNEL_H1", "ts2")
CHUNK_BUFS = int(_os.environ.get("KERNEL_CHUNK_BUFS", "4"))
# how many of the IBLK per-tile h1 column-ops run on ACT instead of DVE
N_H1_ACT = int(_os.environ.get("KERNEL_H1_ACT", "2"))
# dummy PE matmuls issued while input DMAs land (HAM warmup + earlier start)
WARMUP_MM = int(_os.environ.get("KERNEL_WARMUP_MM", "48"))
# PSUM banks for the edge pipeline; the rest go to the LN/node/head pool
EDGE_PS = int(_os.environ.get("KERNEL_EDGE_PS", "8"))
GRP = int(_os.environ.get("KERNEL_GRP", "4"))  # samples per LN/node group


def _split_excess_waits(nc, max_waits=1):
    """walrus in this container rejects >~2 sem waits on one instruction
    (hits the Tile tail drain). Split excess waits onto same-engine NoOps."""
    for f in nc.m.functions:
        for bb in f.blocks:
            insts = list(bb.instructions)
            new_list = []
            changed = False
            for inst in insts:
                si = inst.sync_info
                if si is not None and si.on_wait and len(si.on_wait) > max_waits:
                    waits = list(si.on_wait)
                    extra, keep = waits[:-max_waits], waits[-max_waits:]
                    for k0 in range(0, len(extra), max_waits):
                        chunk = extra[k0 : k0 + max_waits]
                        nop = mybir.InstNoOp(
                            name=f"{inst.name}-wsplit-{k0}",
                            engine=inst.engine,
                            ins=[],
                            outs=[],
                            sync_info=mybir.SyncInfo(on_wait=chunk, on_update=[]),
                        )
                        new_list.append(nop)
                        changed = True
                    si.on_wait = keep
                new_list.append(inst)
            if changed:
                bb.instructions = new_list


def build_bass():
    e_st = BF16 if EDGE_DT == "bf16" else F32
    s_st = BF16 if SMALL_DT == "bf16" else F32
    nc = bass.Bass("TRN2", debug=False, num_devices=NCORES)

    def dp(nm, sh, dt=F32):
        return nc.declare_dram_parameter(nm, sh, dt, isOutput=False)

    obs_d = dp("obsT", [D, TOK], s_st)
    st_d = dp("stateT", [DS, BSH], s_st)
    e1AT_d = dp("e1AT", [D, H], s_st)
    e1CT_d = dp("e1CT", [D, H], s_st)
    e2T_d = dp("e2T", [FT, P, H], e_st)
    e3T_d = dp("e3T", [FT, P, H], e_st)
    n1aT_d = dp("n1aT", [FT, P, H], s_st)
    n1oT_d = dp("n1oT", [D, H], s_st)
    n1sT_d = dp("n1sT", [DS, H], s_st)
    n2T_d = dp("n2T", [FT, P, HH], s_st)
    layerT_d = dp("layerT", [DS, H], s_st)
    mu1T_d = dp("mu1T", [2 * FT, P, 256], s_st)
    s1T_d = dp("s1T", [2 * FT, P, 256], s_st)
    mu2T_d = dp("mu2T", [2, P, 128], s_st)
    s2T_d = dp("s2T", [2, P, 128], s_st)
    mu3T_d = dp("mu3T", [P, NA], s_st)
    s3T_d = dp("s3T", [P, NA], s_st)
    bias_d = dp("bias_pack", [P, 32])
    mu_d = nc.declare_dram_parameter("mu", [NA, BSH], F32, isOutput=True)
    std_d = nc.declare_dram_parameter("std", [NA, BSH], F32, isOutput=True)

    with TileContext(nc) as tc:
        with (
            tc.tile_pool(name="w", bufs=1) as wp,
            tc.tile_pool(name="act", bufs=1) as pa,
            tc.tile_pool(name="chunk", bufs=CHUNK_BUFS) as cp,
            tc.tile_pool(name="ps", bufs=min(EDGE_PS, 8), space="PSUM") as pp,
        ):
            from contextlib import ExitStack as _ES
            _es = _ES()
            if EDGE_PS < 8:
                ppn = _es.enter_context(
                    tc.tile_pool(name="psn", bufs=8 - EDGE_PS, space="PSUM"))
            else:
                ppn = pp
            psn_tag = "psn" if EDGE_PS < 8 else "ps"
            # ---------------- weight loads ----------------
            def wload(nm, dram, idx=None, dt=F32):
                src = dram[:] if idx is None else dram[idx]
                t = wp.tile(list(src.shape), dt, name=nm, tag=nm)
                nc.sync.dma_start(out=t, in_=src)
                return t

            # inputs + first-needed weights first so compute starts early
            o_all = pa.tile([D, TOK], s_st, name="o_all", tag="o_all")
            nc.sync.dma_start(out=o_all, in_=obs_d[:])
            st_t = pa.tile([DS, BSH], s_st, name="st_t", tag="st_t")
            nc.sync.dma_start(out=st_t, in_=st_d[:])
            bias_t = wload("bias_t", bias_d)
            e1Aw = wload("e1Aw", e1AT_d, dt=s_st)
            e1Cw = wload("e1Cw", e1CT_d, dt=s_st)
            layerw = wload("layerw", layerT_d, dt=s_st)
            e2w = [wload(f"e2w{k}", e2T_d, k, e_st) for k in range(FT)]
            e3w = [wload(f"e3w{k}", e3T_d, k, e_st) for k in range(FT)]
            n1aw = [wload(f"n1aw{k}", n1aT_d, k, s_st) for k in range(FT)]
            n1ow = wload("n1ow", n1oT_d, dt=s_st)
            n1sw = wload("n1sw", n1sT_d, dt=s_st)
            n2w = [wload(f"n2w{k}", n2T_d, k, s_st) for k in range(FT)]
            mu1w = [wload(f"mu1w{k}", mu1T_d, k, s_st) for k in range(2 * FT)]
            s1w = [wload(f"s1w{k}", s1T_d, k, s_st) for k in range(2 * FT)]
            mu2w = [wload(f"mu2w{k}", mu2T_d, k, s_st) for k in range(2)]
            s2w = [wload(f"s2w{k}", s2T_d, k, s_st) for k in range(2)]
            mu3w = wload("mu3w", mu3T_d, dt=s_st)
            s3w = wload("s3w", s3T_d, dt=s_st)

            def bcol(i, rows=P):
                return bias_t[0:rows, i : i + 1]

            state_bc = pa.tile([DS, TOK], s_st, name="state_bc", tag="state_bc")
            nc.vector.tensor_copy(
                state_bc[:].rearrange("s (b k) -> s b k", b=BSH),
                st_t[:, :, None].broadcast_to([DS, BSH, K]),
            )

            # PE warmup: keep the tensor engine busy while DMAs land so
            # HAM un-throttles before the real matmul stream starts.
            if WARMUP_MM > 0:
                wdu = pa.tile([P, 64], e_st, name="wdu", tag="wdu")
                nc.vector.memset(wdu, 0.0)
                psd = pp.tile([64, 64], F32, name="psd", tag="ps")
                for _w in range(WARMUP_MM):
                    nc.tensor.matmul(psd, wdu, wdu, start=True, stop=True)

            # st_feat: only needs state + layer weights; run during startup
            xst = []
            for m in range(FT):
                msl = slice(m * P, (m + 1) * P)
                pst = pp.tile([P, BSH], F32, name=f"pst{m}", tag="ps")
                nc.tensor.matmul(pst, layerw[:, msl], st_t, start=True, stop=True)
                xm = pa.tile([P, BSH], s_st, name=f"xst{m}", tag=f"xst{m}")
                nc.scalar.activation(xm, pst, AF.Relu, bias=bcol(16 + m))
                xst.append(xm)

            # ---------------- U/V (edge layer 1, split) ----------------
            U_all, V_all, agg = [], [], []
            for m in range(FT):
                msl = slice(m * P, (m + 1) * P)
                pu = pp.tile([P, TOK], F32, name=f"pu{m}", tag="ps")
                nc.tensor.matmul(pu, e1Aw[:, msl], o_all, start=True, stop=True)
                # U stays f32: tensor_scalar scalar operands must be float32
                Um = pa.tile([P, TOK], F32, name=f"U{m}", tag=f"U{m}")
                nc.scalar.activation(Um, pu, AF.Identity, bias=bcol(0 + m))
                U_all.append(Um)

                pv = pp.tile([P, TOK], F32, name=f"pv{m}", tag="ps")
                nc.tensor.matmul(pv, e1Cw[:, msl], o_all, start=True, stop=True)
                Vm = pa.tile([P, TOK], e_st, name=f"V{m}", tag=f"V{m}")
                nc.vector.tensor_copy(Vm, pv)
                V_all.append(Vm)

                am = pa.tile([P, TOK], F32, name=f"agg{m}", tag=f"agg{m}")
                agg.append(am)

            # LN/node tiles shared across samples
            aggn = []
            hn1 = []
            for m in range(FT):
                anm = pa.tile([P, TOK], s_st, name=f"aggn{m}", tag=f"aggn{m}")
                aggn.append(anm)
                hm = pa.tile([P, TOK], s_st, name=f"hn1_{m}", tag=f"hn1_{m}")
                hn1.append(hm)
            pool_sumf, pool_maxf = [], []
            for m2 in range(HH // P):
                smf = pa.tile([P, BSH], F32, name=f"pool_sf{m2}", tag=f"pool_sf{m2}")
                mmf = pa.tile([P, BSH], F32, name=f"pool_mf{m2}", tag=f"pool_mf{m2}")
                pool_sumf.append(smf)
                pool_maxf.append(mmf)
            ones_col = pa.tile([P, 1], s_st, name="ones_col", tag="ones_col")
            nc.vector.memset(ones_col, 1.0)
            ones_row = pa.tile([1, P], F32, name="ones_row", tag="ones_row")
            nc.vector.memset(ones_row, 1.0)
            eps_t = pa.tile([1, 1], F32, name="eps_t", tag="eps_t")
            nc.vector.memset(eps_t, EPS_S)

            GT = GRP * K

            def ln_node_group(g):
                """LayerNorm + node MLP + pooling for samples [g*GRP,(g+1)*GRP)."""
                gsl = slice(g * GT, (g + 1) * GT)
                sqb, aggb = [], []
                for m in range(FT):
                    sqm = cp.tile([P, GT], s_st, name=f"sq{m}", tag=f"sq_{m}")
                    nc.scalar.activation(sqm, agg[m][:, gsl], AF.Square)
                    sqb.append(sqm)
                    abm = cp.tile([P, GT], s_st, name=f"aggb{m}", tag=f"aggb{m}")
                    nc.vector.tensor_copy(abm, agg[m][:, gsl])
                    aggb.append(abm)
                ps_sum = ppn.tile([1, GT], F32, name="ps_sum", tag=psn_tag)
                ps_ssq = ppn.tile([1, GT], F32, name="ps_ssq", tag=psn_tag)
                for m in range(FT):
                    nc.tensor.matmul(ps_sum, ones_col, aggb[m],
                                     start=(m == 0), stop=(m == FT - 1))
                for m in range(FT):
                    nc.tensor.matmul(ps_ssq, ones_col, sqb[m],
                                     start=(m == 0), stop=(m == FT - 1))
                mean_r = cp.tile([1, GT], F32, name="mean_r", tag="mean_r")
                nc.vector.tensor_scalar_mul(mean_r, ps_sum, 1.0 / H)
                msq_r = cp.tile([1, GT], F32, name="msq_r", tag="msq_r")
                nc.vector.tensor_mul(msq_r, mean_r, mean_r)
                var_r = cp.tile([1, GT], F32, name="var_r", tag="var_r")
                nc.vector.scalar_tensor_tensor(
                    var_r, ps_ssq, 1.0 / H, msq_r, op0=ALU.mult,
                    op1=ALU.subtract)
                # rstd = exp(-0.5*ln(var+eps)): two fast ACT ops instead of
                # Sqrt + slow DVE reciprocal (Rsqrt activation is banned)
                lnv_r = cp.tile([1, GT], F32, name="lnv_r", tag="lnv_r")
                nc.scalar.activation(lnv_r, var_r, AF.Ln, bias=eps_t)
                rstd_r = cp.tile([1, GT], F32, name="rstd_r", tag="rstd_r")
                nc.scalar.activation(rstd_r, lnv_r, AF.Exp, scale=-0.5)

                ps_mb = ppn.tile([P, GT], F32, name="ps_mb", tag=psn_tag)
                nc.tensor.matmul(ps_mb, ones_row, mean_r, start=True, stop=True)
                mean_bc = cp.tile([P, GT], F32, name="mean_bc", tag="mean_bc")
                nc.scalar.copy(mean_bc, ps_mb)
                ps_rb = ppn.tile([P, GT], F32, name="ps_rb", tag=psn_tag)
                nc.tensor.matmul(ps_rb, ones_row, rstd_r, start=True, stop=True)
                rstd_bc = cp.tile([P, GT], F32, name="rstd_bc", tag="rstd_bc")
                nc.scalar.copy(rstd_bc, ps_rb)

                for m in range(FT):
                    tmp = cp.tile([P, GT], F32, name=f"aggt{m}", tag=f"aggt{m}")
                    nc.vector.tensor_sub(tmp, agg[m][:, gsl], mean_bc)
                    nc.vector.tensor_mul(aggn[m][:, gsl], tmp, rstd_bc)

                for m in range(FT):
                    msl = slice(m * P, (m + 1) * P)
                    psn = ppn.tile([P, GT], F32, name=f"psn1_{m}", tag=psn_tag)
                    # LN-independent contributions first: PE can run these
                    # while the LN stats chain (DVE) is still computing
                    nc.tensor.matmul(psn, n1ow[:, msl], o_all[:, gsl],
                                     start=True, stop=False)
                    nc.tensor.matmul(psn, n1sw[:, msl], state_bc[:, gsl],
                                     start=False, stop=False)
                    for k2 in range(FT):
                        nc.tensor.matmul(psn, n1aw[k2][:, msl], aggn[k2][:, gsl],
                                         start=False, stop=(k2 == FT - 1))
                    nc.scalar.activation(hn1[m][:, gsl], psn, AF.Relu,
                                         bias=bcol(12 + m))

                for m2 in range(HH // P):
                    msl = slice(m2 * P, (m2 + 1) * P)
                    psn2 = ppn.tile([P, GT], F32, name=f"psn2_{m2}", tag=psn_tag)
                    for k2 in range(FT):
                        nc.tensor.matmul(psn2, n2w[k2][:, msl], hn1[k2][:, gsl],
                                         start=(k2 == 0), stop=(k2 == FT - 1))
                    nc.vector.reduce_sum(
                        out=pool_sumf[m2][:, g * GRP : (g + 1) * GRP],
                        in_=psn2[:].rearrange("p (b j) -> p b j", b=GRP),
                        axis=AX.X)
                    nc.vector.reduce_max(
                        out=pool_maxf[m2][:, g * GRP : (g + 1) * GRP],
                        in_=psn2[:].rearrange("p (b j) -> p b j", b=GRP),
                        axis=AX.X)

            # ---------------- edge MLP over K x K pairs ----------------
            def build_h1(b, ib):
                """h1[:, (i,j)] = relu(V_j + U_i) for one 8-row i-block."""
                i0 = b * K + ib * IBLK
                h1 = []
                for m in range(FT):
                    h1m = cp.tile([P, IBLK * K], e_st,
                                  name=f"h1_{m}", tag=f"h1_{m}")
                    Vsl = V_all[m][:, b * K : (b + 1) * K]
                    for i in range(IBLK):
                        if i >= IBLK - N_H1_ACT:
                            # offload a slice of h1 work to ACT
                            nc.scalar.activation(
                                h1m[:, i * K : (i + 1) * K], Vsl, AF.Relu,
                                bias=U_all[m][:, i0 + i : i0 + i + 1])
                        else:
                            nc.vector.tensor_scalar(
                                h1m[:, i * K : (i + 1) * K], Vsl,
                                U_all[m][:, i0 + i : i0 + i + 1], 0.0,
                                op0=ALU.add, op1=ALU.max)
                    h1.append(h1m)
                return h1

            chunk_list = [(b, ib) for b in range(BSH) for ib in range(NCH)]
            h1 = build_h1(*chunk_list[0])
            for ci, (b, ib) in enumerate(chunk_list):
                    i0 = b * K + ib * IBLK
                    # emit next chunk's h1 FIRST: DVE is in-order, so queueing
                    # it ahead of this chunk's reduces lets the next L2 phase
                    # start without waiting on this chunk's L3->evict->reduce
                    h1_next = (build_h1(*chunk_list[ci + 1])
                               if ci + 1 < len(chunk_list) else None)
                    h2 = []
                    for m in range(FT):
                        msl = slice(m * P, (m + 1) * P)
                        ps2 = pp.tile([P, IBLK * K], F32, name=f"ps2_{m}", tag="ps")
                        for k2 in range(FT):
                            nc.tensor.matmul(
                                ps2, e2w[k2][:, msl], h1[k2],
                                start=(k2 == 0), stop=(k2 == FT - 1),
                            )
                        h2m = cp.tile([P, IBLK * K], e_st,
                                      name=f"h2_{m}", tag=f"h2_{m}")
                        nc.scalar.activation(h2m, ps2, AF.Relu, bias=bcol(4 + m))
                        h2.append(h2m)
                    for m in range(FT):
                        msl = slice(m * P, (m + 1) * P)
                        ps3 = pp.tile([P, IBLK * K], F32, name=f"ps3_{m}", tag="ps")
                        for k2 in range(FT):
                            nc.tensor.matmul(
                                ps3, e3w[k2][:, msl], h2[k2],
                                start=(k2 == 0), stop=(k2 == FT - 1),
                            )
                        h3m = cp.tile([P, IBLK * K], e_st,
                                      name=f"h3_{m}", tag=f"h3_{m}")
                        if m < N_FUSED_ACT:
                            # fused per-i: relu(x+b) evict with accumulated
                            # j-sum directly into agg (no DVE reduce)
                            for i in range(IBLK):
                                nc.scalar.activation(
                                    h3m[:, i * K : (i + 1) * K],
                                    ps3[:, i * K : (i + 1) * K],
                                    AF.Relu, bias=bcol(8 + m),
                                    accum_out=agg[m][:, i0 + i : i0 + i + 1])
                        else:
                            if m < N_FUSED_ACT + N_EVICT_DVE:
                                nc.vector.tensor_scalar(
                                    h3m, ps3, bcol(8 + m), 0.0,
                                    op0=ALU.add, op1=ALU.max)
                            else:
                                nc.scalar.activation(h3m, ps3, AF.Relu,
                                                     bias=bcol(8 + m))
                            nc.vector.reduce_sum(
                                out=agg[m][:, i0 : i0 + IBLK],
                                in_=h3m[:].rearrange("p (i j) -> p i j", i=IBLK),
                                axis=AX.X,
                            )
                    if ib == NCH - 1 and b % GRP == GRP - 1:
                        ln_node_group(b // GRP)
                    h1 = h1_next

            # ---------------- LN/node interleaved per sample group --------

            pool_sum, pool_max = [], []
            for m2 in range(HH // P):
                sm = pa.tile([P, BSH], s_st, name=f"pool_s{m2}", tag=f"pool_s{m2}")
                nc.vector.tensor_copy(sm, pool_sumf[m2])
                pool_sum.append(sm)
                mm_ = pa.tile([P, BSH], s_st, name=f"pool_m{m2}", tag=f"pool_m{m2}")
                nc.vector.tensor_copy(mm_, pool_maxf[m2])
                pool_max.append(mm_)

            # ---------------- heads ----------------
            xs = xst + pool_sum + pool_max  # x = [st_feat, mean, max]

            def head(w1, w2, w3, bc1, bc2, tag):
                hl1 = []
                for m in range(2):
                    msl = slice(m * P, (m + 1) * P)
                    ph = ppn.tile([P, BSH], F32, name=f"p{tag}1_{m}", tag=psn_tag)
                    for k2 in range(2 * FT):
                        nc.tensor.matmul(ph, w1[k2][:, msl], xs[k2],
                                         start=(k2 == 0), stop=(k2 == 2 * FT - 1))
                    hm = pa.tile([P, BSH], s_st, name=f"h{tag}1_{m}",
                                 tag=f"h{tag}1_{m}")
                    nc.scalar.activation(hm, ph, AF.Relu, bias=bcol(bc1 + m))
                    hl1.append(hm)
                ph2 = ppn.tile([P, BSH], F32, name=f"p{tag}2", tag=psn_tag)
                for k2 in range(2):
                    nc.tensor.matmul(ph2, w2[k2], hl1[k2],
                                     start=(k2 == 0), stop=(k2 == 1))
                hm2 = pa.tile([P, BSH], s_st, name=f"h{tag}2", tag=f"h{tag}2")
                nc.scalar.activation(hm2, ph2, AF.Relu, bias=bcol(bc2))
                ph3 = ppn.tile([NA, BSH], F32, name=f"p{tag}3", tag=psn_tag)
                nc.tensor.matmul(ph3, w3, hm2, start=True, stop=True)
                return ph3

            ph3_mu = head(mu1w, mu2w, mu3w, 20, 22, "mu")
            mu_sb = pa.tile([NA, BSH], F32, name="mu_sb", tag="mu_sb")
            nc.scalar.activation(mu_sb, ph3_mu, AF.Identity, bias=bcol(23, rows=NA))
            nc.sync.dma_start(out=mu_d[:], in_=mu_sb)

            # softplus isn't co-loadable with sqrt in the ACT func sets;
            # emit the pre-softplus logits, host applies softplus+clip.
            ph3_s = head(s1w, s2w, s3w, 24, 26, "s")
            std_sb = pa.tile([NA, BSH], F32, name="std_sb", tag="std_sb")
            nc.scalar.activation(std_sb, ph3_s, AF.Identity, bias=bcol(27, rows=NA))
            nc.sync.dma_start(out=std_d[:], in_=std_sb)

    _split_excess_waits(nc)
    return nc


def prep_weights(inp):
    """Host-side weight preprocessing -> dict of replicated arrays."""
    if SMALL_DT == "bf16" or EDGE_DT == "bf16":
        import ml_dtypes
    s_np = np.float32 if SMALL_DT == "f32" else ml_dtypes.bfloat16
    e_np = np.float32 if EDGE_DT == "f32" else ml_dtypes.bfloat16

    def fs(a):
        return np.ascontiguousarray(np.asarray(a, np.float32), dtype=s_np)

    e1_w = np.asarray(inp["e1_w"], np.float32)
    n1_w = np.asarray(inp["n1_w"], np.float32)
    ln_g = np.asarray(inp["ln_g"], np.float32)
    ln_b = np.asarray(inp["ln_b"], np.float32)
    n2_b = np.asarray(inp["n2_b"], np.float32)
    mu1_w = np.asarray(inp["mu1_w"], np.float32)
    s1_w = np.asarray(inp["s1_w"], np.float32)

    d = {}
    d["e1AT"] = fs(e1_w[:, :D].T)
    d["e1CT"] = fs(e1_w[:, D:].T)
    d["e2T"] = np.ascontiguousarray(
        np.asarray(inp["e2_w"], np.float32).T.reshape(FT, P, H), dtype=e_np)
    d["e3T"] = np.ascontiguousarray(
        np.asarray(inp["e3_w"], np.float32).T.reshape(FT, P, H), dtype=e_np)
    d["n1aT"] = fs((n1_w[:, D : D + H] * ln_g[None, :]).T.reshape(FT, P, H))
    d["n1oT"] = fs(n1_w[:, :D].T)
    d["n1sT"] = fs(n1_w[:, D + H :].T)
    d["n2T"] = fs(np.asarray(inp["n2_w"], np.float32).T.reshape(FT, P, HH))
    d["layerT"] = fs(np.asarray(inp["layer_w"], np.float32).T)

    mu1 = mu1_w.copy()
    mu1[:, H : H + HH] *= 1.0 / K
    d["mu1T"] = fs(mu1.T.reshape(2 * FT, P, 256))
    s1 = s1_w.copy()
    s1[:, H : H + HH] *= 1.0 / K
    d["s1T"] = fs(s1.T.reshape(2 * FT, P, 256))
    d["mu2T"] = fs(np.asarray(inp["mu2_w"], np.float32).T.reshape(2, P, 128))
    d["s2T"] = fs(np.asarray(inp["s2_w"], np.float32).T.reshape(2, P, 128))
    d["mu3T"] = fs(np.asarray(inp["mu3_w"], np.float32).T)
    d["s3T"] = fs(np.asarray(inp["s3_w"], np.float32).T)

    n1_b_eff = np.asarray(inp["n1_b"], np.float32) + n1_w[:, D : D + H] @ ln_b
    mu1_b_eff = (np.asarray(inp["mu1_b"], np.float32)
                 + (mu1_w[:, H : H + HH] + mu1_w[:, H + HH :]) @ n2_b)
    s1_b_eff = (np.asarray(inp["s1_b"], np.float32)
                + (s1_w[:, H : H + HH] + s1_w[:, H + HH :]) @ n2_b)

    bp = np.zeros((P, 32), np.float32)
    bp[:, 0:4] = np.asarray(inp["e1_b"], np.float32).reshape(FT, P).T
    bp[:, 4:8] = np.asarray(inp["e2_b"], np.float32).reshape(FT, P).T
    bp[:, 8:12] = np.asarray(inp["e3_b"], np.float32).reshape(FT, P).T
    bp[:, 12:16] = n1_b_eff.reshape(FT, P).T
    bp[:, 16:20] = np.asarray(inp["layer_b"], np.float32).reshape(FT, P).T
    bp[:, 20:22] = mu1_b_eff.reshape(2, P).T
    bp[:, 22] = np.asarray(inp["mu2_b"], np.float32)
    bp[0:NA, 23] = np.asarray(inp["mu3_b"], np.float32)
    bp[:, 24:26] = s1_b_eff.reshape(2, P).T
    bp[:, 26] = np.asarray(inp["s2_b"], np.float32)
    bp[0:NA, 27] = np.asarray(inp["s3_b"], np.float32)
    d["bias_pack"] = bp
    return d


def make_in_maps(inputs):
    w = prep_weights(inputs)
    if SMALL_DT == "bf16":
        import ml_dtypes
        s_np = ml_dtypes.bfloat16
    else:
        s_np = np.float32
    obs = np.asarray(inputs["obs"], np.float32)
    state = np.asarray(inputs["state"], np.float32)
    in_maps = []
    for c in range(NCORES):
        m = dict(w)
        m["obsT"] = np.ascontiguousarray(
            obs[c * BSH : (c + 1) * BSH].transpose(1, 0, 2).reshape(D, TOK),
            dtype=s_np)
        m["stateT"] = np.ascontiguousarray(
            state[c * BSH : (c + 1) * BSH].T, dtype=s_np)
        in_maps.append(m)
    return in_maps


_NC_CACHE = {}


def get_nc():
    key = (EDGE_DT, SMALL_DT, N_EVICT_DVE, H1_STYLE, CHUNK_BUFS, N_FUSED_ACT, N_H1_ACT, WARMUP_MM, EDGE_PS, GRP)
    if key not in _NC_CACHE:
        _NC_CACHE[key] = build_bass()
    return _NC_CACHE[key]


def run(in_maps, trace=False, **kw):
    nc = get_nc()
    return run_bass_kernel_spmd(nc, in_maps, core_ids=list(range(NCORES)),
                                trace=trace, **kw)


def gather(res_list):
    mu = np.concatenate([r["mu"].T for r in res_list], axis=0)
    pre = np.concatenate([r["std"].T for r in res_list], axis=0).astype(np.float64)
    std = np.clip(np.log1p(np.exp(pre)) + 0.001, 0.1, 2.0)
    return mu.astype(np.float32), std.astype(np.float32)


def kernel(**inputs):
    res = run(make_in_maps(inputs))
    return gather(res.results)
